# revision 1
# baseline (speedup 1.0000x reference)
"""Multi-head attention on 8 TRN2 NeuronCores (Bass/Tile, SPMD).

Sharding: tensor-parallel over heads (2 heads/core) for qkv + attention,
then AllToAll to token-sharded layout for the output projection
(each core produces the final output for 1/8 of the tokens).

Layouts (per core):
  xT      [D, TOK]   f32  -- x transposed (host-prepped), replicated
  wqkv    [D, 384]   f32  -- [qA qB | kA kB | vA vB] columns for this core's heads
  bqkv    [128, 3]   f32  -- per-partition bias, col j for col-tile j (q,k,v)
  wproj   [D, D]     bf16 -- replicated, rows grouped by source core
  bproj   [128, 8]   f32  -- col j = bias for out-dim tile j
  out     [D, TS]    f32  -- projected output, transposed, for token slice

Inside: q^T,k^T [128(2 heads x 64dh), TOK] f32 in SBUF; v transposed back to
natural [k, dh] layout (PE transpose) with a ones column appended so the
PV matmul also produces softmax denominators (row 64).  Scores are computed
transposed (S^T[k,q]) so no attn transpose is needed for PV.  exp() without
max-subtraction (scores are ~N(0,1) after 1/sqrt(dh) scaling -- bounded).
"""

import numpy as np
import ml_dtypes

import concourse.bass as bass
import concourse.mybir as mybir
import concourse.tile as tile
from concourse import bacc
from concourse.bass_utils import run_bass_kernel_spmd
from concourse.masks import make_identity

# Route `exp` activations to the natural_log_exp_and_others table set so
# exp and ln share one ACT table load (otherwise the table pass thrashes
# 2.7us loads between exp_and_others and the ln set on every softmax
# normalize).  Keeps dict order so act_func_set_id indices stay valid.
import concourse.bacc as _bacc_mod
from concourse.hw_specs import get_activation_tables as _orig_gat


def _gat_exp_with_ln(arch):
    d = dict(_orig_gat(arch))
    for name in d:
        if "exp" in name and "natural_log" not in name:
            d[name] = d[name] - {mybir.ActivationFunctionType.Exp}
    return d


_bacc_mod.get_activation_tables = _gat_exp_with_ln

# problem dims (fixed by the harness contract)
B, T, D, H = 4, 2048, 1024, 16
DH = D // H          # 64
NCORES = 8
HPC = H // NCORES    # 2 heads per core

F32 = mybir.dt.float32
F32R = mybir.dt.float32r
BF16 = mybir.dt.bfloat16
FP16 = mybir.dt.float16
EXP = mybir.ActivationFunctionType.Exp
LOG = mybir.ActivationFunctionType.Ln


def emit(tc, io, B_, T_, phases="full"):
    """Emit the per-core program. io: dict of DRAM APs."""
    nc = tc.nc
    TOK = B_ * T_
    TS = TOK // NCORES        # output token slice per core
    NCH = TOK // 512          # 512-token chunks (qkv streaming)
    NQC = T_ // 512           # q-chunks per batch
    NKT = T_ // 128           # k-tiles per batch
    CPB = T_ // 512           # chunks per batch

    xT, wqkv, bqkv, wproj, bproj, out = (
        io["xT"], io["wqkv"], io["bqkv"], io["wproj"], io["bproj"], io["out"])

    with tc.tile_pool(name="consts", bufs=1) as consts, \
         tc.tile_pool(name="bigs", bufs=1) as bigs, \
         tc.tile_pool(name="dram", bufs=1, space="DRAM") as dram:
        # ---- constants ----
        w_sb = consts.tile([128, 8, 384], FP16)
        for d in range(8):
            nc.sync.dma_start(out=w_sb[:, d, :], in_=wqkv[d * 128:(d + 1) * 128, :])
        bias_sb = consts.tile([128, 3], F32)
        nc.sync.dma_start(out=bias_sb, in_=bqkv)
        bproj_sb = consts.tile([128, 8], F32)
        nc.sync.dma_start(out=bproj_sb, in_=bproj)
        wproj_sb = consts.tile([128, 8, 1024], FP16)
        for d in range(8):
            nc.sync.dma_start(out=wproj_sb[:, d, :], in_=wproj[d * 128:(d + 1) * 128, :])
        ident = consts.tile([128, 128], F32)
        make_identity(nc, ident)
        # band masks: mask[j][pk, fq] = 1 if pk + 128*j <= fq else 0
        masks = consts.tile([128, 4, 512], FP16)
        nc.vector.memset(masks, 1.0)
        for j in range(4):
            nc.gpsimd.affine_select(
                out=masks[:, j, :], in_=masks[:, j, :],
                compare_op=mybir.AluOpType.is_ge, fill=0.0,
                base=-128 * j, pattern=[[1, 512]], channel_multiplier=-1)

        # ---- big persistent buffers ----
        qT_sb = bigs.tile([128, TOK], FP16)
        kT_sb = bigs.tile([128, TOK], FP16)
        vn_sb = bigs.tile([128, B_, HPC, NKT, 65], FP16)
        for b in range(B_):
            for hh in range(HPC):
                ones_bcast = bass.AP(
                    tensor=io["ones"].tensor, offset=0,
                    ap=[[1, 128], [0, NKT], [0, 1]])
                nc.sync.dma_start(out=vn_sb[:, b, hh, :, 64:65], in_=ones_bcast)

        a2a_in = dram.tile([NCORES, 128, TS], FP16)
        a2a_out = dram.tile([NCORES, 128, TS], FP16)

        if phases == "a0":
            with tc.tile_pool(name="dbg", bufs=1) as dbg:
                st = dbg.tile([128, TS], F32)
                nc.vector.tensor_copy(st, masks[:, 0, 0:TS])
                nc.sync.dma_start(out=out[0:128, :], in_=st)
                st2 = dbg.tile([128, TS], F32)
                nc.vector.tensor_copy(st2, w_sb[:, 0, 0:TS])
                nc.sync.dma_start(out=out[128:256, :], in_=st2)
                st3 = dbg.tile([128, TS], F32)
                nc.vector.tensor_copy(st3, ident[:, 0:TS] if TS <= 128 else ident[:, 0:128].broadcast_to((128, TS)))
                nc.sync.dma_start(out=out[256:384, :], in_=st3[:, 0:TS])
            return

        # ---- phases A+B merged: per-batch qkv -> attention pipeline ----
        with tc.tile_pool(name="xt", bufs=3) as xt_pool, \
             tc.tile_pool(name="vstage", bufs=2) as vstage_pool, \
             tc.tile_pool(name="psall", bufs=1, space="PSUM") as ps_all, \
             tc.tile_pool(name="expp", bufs=3) as exp_pool, \
             tc.tile_pool(name="attp", bufs=3) as att_pool, \
             tc.tile_pool(name="rcp", bufs=2) as rc_pool:

            def qkv_batch(b):
                for ci in range(CPB):
                    ch = b * CPB + ci
                    tt0 = ci * 4
                    xt = xt_pool.tile([128, 8, 512], FP16, tag="xt", name=f"xt{ch}")
                    for d in range(8):
                        nc.sync.dma_start(
                            out=xt[:, d, :],
                            in_=xT[d * 128:(d + 1) * 128, ch * 512:(ch + 1) * 512])
                    for ct in range(3):
                        ps = ps_all.tile([128, 512], F32, tag="qkvps", bufs=2,
                                         name=f"qkvps{ch}_{ct}")
                        for d in range(8):
                            nc.tensor.matmul(
                                ps,
                                w_sb[:, d, ct * 128:(ct + 1) * 128],
                                xt[:, d, :],
                                start=(d == 0), stop=(d == 7))
                        if ct == 0:
                            nc.vector.tensor_scalar_add(
                                qT_sb[:, ch * 512:(ch + 1) * 512], ps, bias_sb[:, 0:1])
                        elif ct == 1:
                            nc.vector.tensor_scalar_add(
                                kT_sb[:, ch * 512:(ch + 1) * 512], ps, bias_sb[:, 1:2])
                        else:
                            vst = vstage_pool.tile([128, 512], F32, tag="vst",
                                                   name=f"vst{ch}")
                            nc.vector.tensor_scalar_add(vst, ps, bias_sb[:, 2:3])
                            vtp = ps_all.tile([128, 512], F32, tag="qkvps", bufs=2,
                                              name=f"vtps{ch}")
                            for sub in range(4):
                                nc.tensor.transpose(
                                    vtp[:, sub * 128:(sub + 1) * 128],
                                    vst[:, sub * 128:(sub + 1) * 128],
                                    ident)
                            for hh in range(HPC):
                                nc.vector.tensor_copy(
                                    vn_sb[:, b, hh, tt0:tt0 + 4, 0:64],
                                    vtp.rearrange("p (s x) -> p s x", s=4)[
                                        :, :, hh * 64:(hh + 1) * 64])

            def attention_batch(b):
                for hh in range(HPC):
                    p0 = hh * 64
                    for qc in range(NQC):
                        nkt_q = 4 * qc + 4
                        q_sl = qT_sb[p0:p0 + 64,
                                     b * T_ + qc * 512: b * T_ + (qc + 1) * 512]
                        pv = ps_all.tile([128, 512], F32, tag="pv", bufs=2,
                                         name=f"pv{b}_{hh}_{qc}")
                        for g in range(nkt_q // 2):
                            sc = ps_all.tile([128, 1024], F32, tag="sc", bufs=2,
                                             name=f"sc{b}_{hh}_{qc}_{g}")
                            for j in range(2):
                                kt = 2 * g + j
                                nc.tensor.matmul(
                                    sc[:, j * 512:(j + 1) * 512],
                                    kT_sb[p0:p0 + 64,
                                          b * T_ + kt * 128: b * T_ + (kt + 1) * 128],
                                    q_sl, start=True, stop=True)
                            ex = exp_pool.tile([128, 1024], FP16, tag="ex",
                                               name=f"ex{b}_{hh}_{qc}_{g}")
                            nc.scalar.activation(ex, sc, EXP, scale=0.125)
                            for j in range(2):
                                kt = 2 * g + j
                                bj = kt - 4 * qc
                                if bj >= 0:
                                    nc.vector.tensor_mul(
                                        ex[:, j * 512:(j + 1) * 512],
                                        ex[:, j * 512:(j + 1) * 512],
                                        masks[:, bj, :])
                            for j in range(2):
                                kt = 2 * g + j
                                nc.tensor.matmul(
                                    pv[0:65, :],
                                    vn_sb[:, b, hh, kt, :],
                                    ex[:, j * 512:(j + 1) * 512],
                                    start=(kt == 0), stop=(kt == nkt_q - 1))
                        lg = rc_pool.tile([1, 512], F32, tag="lg",
                                          name=f"lg{b}_{hh}_{qc}")
                        nc.scalar.activation(lg, pv[64:65, :], LOG)
                        rc = rc_pool.tile([1, 512], F32, tag="rc",
                                          name=f"rc{b}_{hh}_{qc}")
                        nc.scalar.activation(rc, lg, EXP, scale=-1.0)
                        rcb = rc_pool.tile([64, 512], F32, tag="rcb",
                                           name=f"rcb{b}_{hh}_{qc}")
                        nc.gpsimd.partition_broadcast(rcb, rc)
                        at = att_pool.tile([64, 512], FP16, tag="at",
                                           name=f"at{b}_{hh}_{qc}")
                        nc.vector.tensor_mul(at, pv[0:64, :], rcb)
                        tok0 = b * T_ + qc * 512
                        if TS >= 512:
                            jj, c0 = tok0 // TS, tok0 % TS
                            dst = a2a_in[jj, p0:p0 + 64, c0:c0 + 512]
                        else:
                            nj = 512 // TS
                            j0 = tok0 // TS
                            dst = a2a_in[j0:j0 + nj, p0:p0 + 64, :].rearrange(
                                "j p c -> p j c")
                        nc.sync.dma_start(out=dst, in_=at)

            qkv_batch(0)
            for b in range(B_):
                attention_batch(b)
                if b + 1 < B_:
                    qkv_batch(b + 1)

        # ---- phase C: AllToAll ----
        nc.gpsimd.collective_compute(
            "AllToAll", mybir.AluOpType.bypass,
            replica_groups=[list(range(NCORES))],
            ins=[a2a_in[:]], outs=[a2a_out[:]])

        if phases == "abc":
            with tc.tile_pool(name="dbg", bufs=1) as dbg:
                st0 = dbg.tile([128, TS], FP16)
                nc.sync.dma_start(out=st0, in_=a2a_out[0])
                st = dbg.tile([128, TS], F32)
                nc.vector.tensor_copy(st, st0)
                nc.sync.dma_start(out=out[0:128, :], in_=st)
            return

        # ---- phase D: projection on own token slice ----
        W = min(512, TS)
        NOC = (TS + 511) // 512
        with tc.tile_pool(name="rhsp", bufs=1) as rhs_pool, \
             tc.tile_pool(name="pjps", bufs=4, space="PSUM") as pj_ps, \
             tc.tile_pool(name="otp", bufs=3) as out_pool:
            rhs_sb = rhs_pool.tile([128, 8, TS], FP16)
            for i in range(8):
                nc.sync.dma_start(out=rhs_sb[:, i, :], in_=a2a_out[i])
            for od in range(8):
                pjs = []
                for c in range(NOC):
                    pjs.append(pj_ps.tile([128, W], F32, tag="pjps",
                                          name=f"pj{od}_{c}"))
                for i in range(8):
                    for c in range(NOC):
                        nc.tensor.matmul(
                            pjs[c],
                            wproj_sb[:, i, od * 128:(od + 1) * 128],
                            rhs_sb[:, i, c * W:(c + 1) * W],
                            start=(i == 0), stop=(i == 7))
                for c in range(NOC):
                    ot = out_pool.tile([128, W], F32, tag="ot", name=f"ot{od}_{c}")
                    nc.vector.tensor_scalar_add(ot, pjs[c], bproj_sb[:, od:od + 1])
                    nc.sync.dma_start(
                        out=out[od * 128:(od + 1) * 128, c * W:(c + 1) * W], in_=ot)


def build_nc(B_=B, T_=T, phases="full"):
    TOK = B_ * T_
    TS = TOK // NCORES
    nc = bacc.Bacc("TRN2", target_bir_lowering=False, debug=False,
                   enable_asserts=False, num_devices=NCORES)
    io = {
        "xT": nc.dram_tensor("xT", [D, TOK], FP16, kind="ExternalInput").ap(),
        "wqkv": nc.dram_tensor("wqkv", [D, 3 * HPC * DH], FP16, kind="ExternalInput").ap(),
        "bqkv": nc.dram_tensor("bqkv", [128, 3], F32, kind="ExternalInput").ap(),
        "wproj": nc.dram_tensor("wproj", [D, D], FP16, kind="ExternalInput").ap(),
        "bproj": nc.dram_tensor("bproj", [128, 8], F32, kind="ExternalInput").ap(),
        "ones": nc.dram_tensor("ones", [128, 1], FP16, kind="ExternalInput").ap(),
        "out": nc.dram_tensor("out", [D, TS], F32, kind="ExternalOutput").ap(),
    }
    with tile.TileContext(nc) as tc:
        emit(tc, io, B_, T_, phases)
    nc.compile()
    return nc


def make_in_maps(x, W_qkv, b_qkv, W_proj, b_proj, B_=B, T_=T):
    """Shard host inputs per core."""
    TOK = B_ * T_
    x2 = np.asarray(x, np.float32).reshape(TOK, D)
    xT = np.ascontiguousarray(x2.T).astype(np.float16)   # [D, TOK]
    wproj_bf = np.asarray(W_proj, np.float32).astype(np.float16)
    bproj_rs = np.ascontiguousarray(
        np.asarray(b_proj, np.float32).reshape(8, 128).T)  # [128, 8]
    in_maps = []
    for c in range(NCORES):
        cols = []
        bcols = []
        for part in range(3):                            # q, k, v
            for h in (2 * c, 2 * c + 1):
                sl = slice(part * D + h * DH, part * D + (h + 1) * DH)
                cols.append(np.asarray(W_qkv, np.float32)[:, sl])
                bcols.append(np.asarray(b_qkv, np.float32)[sl])
        wq = np.ascontiguousarray(np.concatenate(cols, axis=1)).astype(np.float16)
        bq = np.ascontiguousarray(
            np.concatenate(bcols).reshape(3, 128).T)                 # [128, 3]
        in_maps.append({
            "xT": xT, "wqkv": wq, "bqkv": bq,
            "wproj": wproj_bf, "bproj": bproj_rs,
            "ones": np.ones((128, 1), np.float16),
        })
    return in_maps


def gather_out(results, B_=B, T_=T):
    TOK = B_ * T_
    TS = TOK // NCORES
    fullT = np.empty((D, TOK), np.float32)
    for c in range(NCORES):
        fullT[:, c * TS:(c + 1) * TS] = results[c]["out"]
    return np.ascontiguousarray(fullT.T).reshape(B_, T_, D)


_NC_CACHE = {}
LAST_EXEC_NS = None


def kernel(x, mask, W_qkv, b_qkv, W_proj, b_proj, trace=False):
    global LAST_EXEC_NS
    key = (B, T)
    if key not in _NC_CACHE:
        _NC_CACHE[key] = build_nc(B, T)
    nc = _NC_CACHE[key]
    in_maps = make_in_maps(x, W_qkv, b_qkv, W_proj, b_proj, B, T)
    res = run_bass_kernel_spmd(nc, in_maps, core_ids=list(range(NCORES)),
                               trace=trace)
    LAST_EXEC_NS = res.exec_time_ns
    return gather_out(res.results, B, T)



# revision 3
# speedup vs baseline: 1.2738x; 1.2738x over previous
"""Multi-head attention on 8 TRN2 NeuronCores (Bass/Tile, SPMD).

Sharding: tensor-parallel over heads (2 heads/core) for qkv + attention;
per-batch AllToAll to token-sharded layout (each core gets a 256-token
slice of every batch, all 16 heads) for the output projection.

Key structure (v2):
- qkv: x^T streamed in 1024-token tiles; q,k -> qT/kT [64dh x TOK] fp16 in
  SBUF (2 heads stacked on partitions); v transposed via PE into
  vn [ktok, vA|ones|vB] tiles so the PV matmul also emits softmax
  denominators (ones column shared by both heads).
- attention: scores computed transposed (S^T[k,q]); exp without max
  subtraction; causal handled by skipping k-tiles above the diagonal and
  shrinking matmuls/exp to the live q-range on diagonal tiles; inner loop
  software-pipelined (QK runs 2 tiles ahead of PV).
- normalization deferred past the collective: unnormalized PV rows plus
  denominator rows ship through a per-batch AllToAll [8,130,256]; the
  receiving core computes 1/denom once (ln/exp), broadcasts it across
  partitions with a tiny select-matmul, scales, projects, adds bias.
- k-bias dropped (softmax-invariant given q-bias); v-bias folded into the
  projection bias host-side.
"""

import math

import numpy as np

import concourse.bass as bass
import concourse.mybir as mybir
import concourse.tile as tile
from concourse import bacc
from concourse.bass_utils import run_bass_kernel_spmd
from concourse.masks import make_identity

# Route `exp` activations to the natural_log_exp_and_others table set so
# exp and ln share one ACT table load.
import concourse.bacc as _bacc_mod
from concourse.hw_specs import get_activation_tables as _orig_gat


def _gat_exp_with_ln(arch):
    d = dict(_orig_gat(arch))
    for name in d:
        if "exp" in name and "natural_log" not in name:
            d[name] = d[name] - {mybir.ActivationFunctionType.Exp}
    return d


_bacc_mod.get_activation_tables = _gat_exp_with_ln

# problem dims (fixed by the harness contract)
B, T, D, H = 4, 2048, 1024, 16
DH = D // H          # 64
NCORES = 8
HPC = H // NCORES    # 2 heads per core

F32 = mybir.dt.float32
FP16 = mybir.dt.float16
EXP = mybir.ActivationFunctionType.Exp
LOG = mybir.ActivationFunctionType.Ln


def emit(tc, io, B_, T_):
    nc = tc.nc
    TOK = B_ * T_
    CPB = T_ // 512           # 512-token compute chunks per batch
    NKT = T_ // 128           # k-tiles per batch
    SL = T_ // NCORES         # per-core token slice of each batch (256)

    xT, wqkv, bq, wproj, bproj, sel, out = (
        io["xT"], io["wqkv"], io["bq"], io["wproj"], io["bproj"],
        io["sel"], io["out"])

    with tc.tile_pool(name="consts", bufs=1) as consts, \
         tc.tile_pool(name="bigs", bufs=1) as bigs, \
         tc.tile_pool(name="dram", bufs=1, space="DRAM") as dram, \
         tc.tile_pool(name="xt", bufs=2) as xt_pool, \
         tc.tile_pool(name="vst", bufs=2) as vst_pool, \
         tc.tile_pool(name="ps", bufs=1, space="PSUM") as ps, \
         tc.tile_pool(name="expp", bufs=4) as ex_pool, \
         tc.tile_pool(name="attp", bufs=3) as at_pool, \
         tc.tile_pool(name="rhsp", bufs=2) as rhs_pool, \
         tc.tile_pool(name="denp", bufs=2) as den_pool, \
         tc.tile_pool(name="otp", bufs=3) as out_pool:

        # ---- constants (front: only what qkv batch 0 needs) ----
        w_sb = consts.tile([128, 8, 384], FP16)
        for d in range(8):
            nc.sync.dma_start(out=w_sb[:, d, :], in_=wqkv[d * 128:(d + 1) * 128, :])
        bq_sb = consts.tile([128, 1], F32)
        nc.sync.dma_start(out=bq_sb, in_=bq)
        sel_sb = consts.tile([16, 8, 128], FP16)
        nc.sync.dma_start(out=sel_sb, in_=sel)
        ident = consts.tile([128, 128], FP16)
        make_identity(nc, ident)
        # band masks: mask[j][pk, fq] = 1 if pk + 128*j <= fq else 0
        masks = consts.tile([128, 4, 512], FP16)
        nc.vector.memset(masks, 1.0)
        for j in range(4):
            nc.gpsimd.affine_select(
                out=masks[:, j, :], in_=masks[:, j, :],
                compare_op=mybir.AluOpType.is_ge, fill=0.0,
                base=-128 * j, pattern=[[1, 512]], channel_multiplier=-1)
        # deferred consts (needed from proj(0) on; loaded mid-stream)
        wproj_sb = consts.tile([128, 8, 1024], FP16)
        bproj_sb = consts.tile([128, 8], F32)

        # ---- big persistent buffers ----
        qT_sb = bigs.tile([128, TOK], FP16)
        kT_sb = bigs.tile([128, TOK], FP16)
        # vn: per k-tile [vA(64) | ones(1) | vB(64) | pad]
        vn_sb = bigs.tile([128, B_, NKT, 130], FP16)
        nc.vector.memset(vn_sb[:, :, :, 64:65], 1.0)

        a2a_in = [dram.tile([NCORES, 130, SL], FP16, name=f"a2ai{b}")
                  for b in range(B_)]
        a2a_out = [dram.tile([NCORES, 130, SL], FP16, name=f"a2ao{b}")
                   for b in range(B_)]

        def qkv_chunk(b, ci, xt_box):
            """One 512-token chunk of q/k/v for batch b."""
            ch = b * CPB + ci
            if ci % 2 == 0:
                xt_box[0] = xt_pool.tile([128, 8, 1024], FP16, tag="xt",
                                         name=f"xt{ch}")
                for d in range(8):
                    nc.sync.dma_start(
                        out=xt_box[0][:, d, :],
                        in_=xT[d * 128:(d + 1) * 128, ch * 512:(ch + 2) * 512])
            xt2 = xt_box[0]
            off = (ci % 2) * 512
            # q then k
            for ct, dst_is_q in ((0, True), (1, False)):
                qk = ps.tile([128, 512], F32, tag="qk", bufs=2,
                             name=f"qk{ch}_{ct}")
                for d in range(8):
                    nc.tensor.matmul(
                        qk, w_sb[:, d, ct * 128:(ct + 1) * 128],
                        xt2[:, d, off:off + 512],
                        start=(d == 0), stop=(d == 7))
                if dst_is_q:
                    nc.vector.tensor_scalar_add(
                        qT_sb[:, ch * 512:(ch + 1) * 512], qk, bq_sb[:, 0:1])
                else:
                    nc.vector.tensor_copy(
                        kT_sb[:, ch * 512:(ch + 1) * 512], qk)
            # v: [vdim, tok] -> SBUF fp16 -> PE transpose -> vn [tok, vdim]
            vp = ps.tile([128, 512], F32, tag="qk", bufs=2, name=f"vp{ch}")
            for d in range(8):
                nc.tensor.matmul(
                    vp, w_sb[:, d, 256:384], xt2[:, d, off:off + 512],
                    start=(d == 0), stop=(d == 7))
            vst = vst_pool.tile([128, 512], FP16, tag="vst", name=f"vst{ch}")
            nc.vector.tensor_copy(vst, vp)
            vtp = ps.tile([128, 1024], FP16, tag="vtp", bufs=1,
                          name=f"vtp{ch}")
            for sub in range(4):
                nc.tensor.transpose(
                    vtp[:, sub * 128:(sub + 1) * 128],
                    vst[:, sub * 128:(sub + 1) * 128], ident)
            t0 = ci * 4
            vtp4 = vtp[:, 0:512].rearrange("p (s x) -> p s x", s=4)
            nc.vector.tensor_copy(vn_sb[:, b, t0:t0 + 4, 0:64],
                                  vtp4[:, :, 0:64])
            nc.vector.tensor_copy(vn_sb[:, b, t0:t0 + 4, 65:129],
                                  vtp4[:, :, 64:128])

        def attention_unit(b, hh, qc):
            p0 = hh * 64
            nkt = 4 * qc + 4
            tb = b * T_

            def ncols(kt):
                return 512 - max(0, 128 * (kt - 4 * qc))

            q_ap = qT_sb[p0:p0 + 64, tb + qc * 512: tb + (qc + 1) * 512]
            pv = ps.tile([128, 512], F32, tag="pv", bufs=2,
                         name=f"pv{b}_{hh}_{qc}")
            scs = {}
            exs = {}

            def emit_qk(kt):
                n = ncols(kt)
                s = 512 - n
                sc = ps.tile([128, 512], F32, tag="sc", bufs=3,
                             name=f"sc{b}_{hh}_{qc}_{kt}")
                nc.tensor.matmul(
                    sc[:, s:512],
                    kT_sb[p0:p0 + 64, tb + kt * 128: tb + (kt + 1) * 128],
                    q_ap[:, s:512], start=True, stop=True)
                scs[kt] = sc

            emit_qk(0)
            if nkt > 1:
                emit_qk(1)
            for kt in range(nkt):
                n = ncols(kt)
                s = 512 - n
                ex = ex_pool.tile([128, 512], FP16, tag="ex",
                                  name=f"ex{b}_{hh}_{qc}_{kt}")
                nc.scalar.activation(ex[:, s:512], scs[kt][:, s:512], EXP,
                                     scale=1.0 / math.sqrt(DH))
                bj = kt - 4 * qc
                if bj >= 0:
                    nc.vector.tensor_mul(ex[:, s:512], ex[:, s:512],
                                         masks[:, bj, s:512])
                exs[kt] = ex
                del scs[kt]
                if kt + 2 < nkt:
                    emit_qk(kt + 2)
                vcol = hh * 64   # [vA|ones] or [ones|vB]
                nc.tensor.matmul(
                    pv[0:65, s:512], vn_sb[:, b, kt, vcol:vcol + 65],
                    ex[:, s:512], start=(kt == 0), stop=(kt == nkt - 1))
                del exs[kt]

            at = at_pool.tile([65, 512], FP16, tag="at",
                              name=f"at{b}_{hh}_{qc}")
            nc.vector.tensor_copy(at, pv[0:65, :])
            # rows: hh=0 -> data 0:64, denom 64; hh=1 -> denom 0, data 1:65
            drow = 64 if hh == 0 else 0
            d0 = 0 if hh == 0 else 1
            j0 = 2 * qc
            dst = a2a_in[b][j0:j0 + 2, p0:p0 + 64, :].rearrange(
                "j p c -> p j c")
            nc.sync.dma_start(
                out=dst,
                in_=at[d0:d0 + 64, :].rearrange("p (j c) -> p j c", j=2))
            ddst = a2a_in[b][j0:j0 + 2, 128 + hh:129 + hh, :].rearrange(
                "j one c -> one j c")
            nc.gpsimd.dma_start(
                out=ddst,
                in_=at[drow:drow + 1, :].rearrange("p (j c) -> p j c", j=2))

        def emit_a2a(b):
            nc.gpsimd.collective_compute(
                "AllToAll", mybir.AluOpType.bypass,
                replica_groups=[list(range(NCORES))],
                ins=[a2a_in[b][:]], outs=[a2a_out[b][:]])

        def proj_batch(b):
            rhs = rhs_pool.tile([128, 8, SL], FP16, tag="rhs",
                                name=f"rhs{b}")
            for i in range(8):
                nc.sync.dma_start(out=rhs[:, i, :], in_=a2a_out[b][i, 0:128, :])
            den = den_pool.tile([16, SL], FP16, tag="den", name=f"den{b}")
            for h in range(2):
                nc.sync.dma_start(out=den[h * 8:(h + 1) * 8, :],
                                  in_=a2a_out[b][:, 128 + h, :])
            lg = den_pool.tile([16, SL], F32, tag="lg", name=f"lg{b}")
            nc.scalar.activation(lg, den, LOG)
            rec = den_pool.tile([16, SL], FP16, tag="rec", name=f"rec{b}")
            nc.scalar.activation(rec, lg, EXP, scale=-1.0)
            for i in range(8):
                rcb = ps.tile([128, 512], F32, tag="qk", bufs=2,
                              name=f"rcb{b}_{i}")
                nc.tensor.matmul(rcb[:, 0:SL], sel_sb[:, i, :], rec,
                                 start=True, stop=True)
                nc.vector.tensor_mul(rhs[:, i, :], rhs[:, i, :], rcb[:, 0:SL])
            for od in range(8):
                pj = ps.tile([128, 512], F32, tag="sc", bufs=3,
                             name=f"pj{b}_{od}")
                for i in range(8):
                    nc.tensor.matmul(
                        pj[:, 0:SL],
                        wproj_sb[:, i, od * 128:(od + 1) * 128],
                        rhs[:, i, :], start=(i == 0), stop=(i == 7))
                ot = out_pool.tile([128, SL], F32, tag="ot",
                                   name=f"ot{b}_{od}")
                nc.vector.tensor_scalar_add(ot, pj[:, 0:SL],
                                            bproj_sb[:, od:od + 1])
                nc.gpsimd.dma_start(
                    out=out[od * 128:(od + 1) * 128, b * SL:(b + 1) * SL],
                    in_=ot)

        # ---- schedule ----
        xt_box = [None]
        for ci in range(CPB):
            qkv_chunk(0, ci, xt_box)
        # deferred const loads (issue while batch-0 attention runs)
        for d in range(8):
            nc.sync.dma_start(out=wproj_sb[:, d, :],
                              in_=wproj[d * 128:(d + 1) * 128, :])
        nc.sync.dma_start(out=bproj_sb, in_=bproj)

        units = [(hh, qc) for hh in range(HPC) for qc in range(CPB)]
        for b in range(B_):
            for u, (hh, qc) in enumerate(units):
                attention_unit(b, hh, qc)
                if b + 1 < B_ and u % 2 == 1:
                    qkv_chunk(b + 1, u // 2, xt_box)
                if u == 3 and b >= 1:
                    proj_batch(b - 1)
            emit_a2a(b)
        proj_batch(B_ - 1)


def build_nc(B_=B, T_=T):
    TOK = B_ * T_
    SL = T_ // NCORES
    nc = bacc.Bacc("TRN2", target_bir_lowering=False, debug=False,
                   enable_asserts=False, num_devices=NCORES)
    io = {
        "xT": nc.dram_tensor("xT", [D, TOK], FP16, kind="ExternalInput").ap(),
        "wqkv": nc.dram_tensor("wqkv", [D, 3 * HPC * DH], FP16,
                               kind="ExternalInput").ap(),
        "bq": nc.dram_tensor("bq", [128, 1], F32, kind="ExternalInput").ap(),
        "wproj": nc.dram_tensor("wproj", [D, D], FP16,
                                kind="ExternalInput").ap(),
        "bproj": nc.dram_tensor("bproj", [128, 8], F32,
                                kind="ExternalInput").ap(),
        "sel": nc.dram_tensor("sel", [16, 8 * 128], FP16,
                              kind="ExternalInput").ap(),
        "out": nc.dram_tensor("out", [D, B_ * SL], F32,
                              kind="ExternalOutput").ap(),
    }
    io["sel"] = io["sel"].rearrange("p (i m) -> p i m", i=8)
    with tile.TileContext(nc) as tc:
        emit(tc, io, B_, T_)
    nc.compile()
    return nc


def make_in_maps(x, W_qkv, b_qkv, W_proj, b_proj, B_=B, T_=T):
    """Shard host inputs per core."""
    TOK = B_ * T_
    x2 = np.asarray(x, np.float32).reshape(TOK, D)
    xT = np.ascontiguousarray(x2.T).astype(np.float16)   # [D, TOK]
    W_qkv = np.asarray(W_qkv, np.float32)
    b_qkv = np.asarray(b_qkv, np.float32)
    W_proj = np.asarray(W_proj, np.float32)
    wproj_16 = W_proj.astype(np.float16)
    b_v = b_qkv[2 * D:3 * D]
    bproj_eff = np.asarray(b_proj, np.float32) + b_v @ W_proj
    bproj_rs = np.ascontiguousarray(bproj_eff.reshape(8, 128).T)  # [128, 8]
    # sel[q, i, m] = 1 iff q == (m // 64) * 8 + i
    sel = np.zeros((16, 8, 128), np.float16)
    for i in range(8):
        sel[i, i, 0:64] = 1.0
        sel[8 + i, i, 64:128] = 1.0
    sel = sel.reshape(16, 8 * 128)
    in_maps = []
    for c in range(NCORES):
        cols = []
        bqc = []
        for part in range(3):                            # q, k, v
            for h in (2 * c, 2 * c + 1):
                sl = slice(part * D + h * DH, part * D + (h + 1) * DH)
                cols.append(W_qkv[:, sl])
                if part == 0:
                    bqc.append(b_qkv[sl])
        wq = np.ascontiguousarray(np.concatenate(cols, axis=1)).astype(
            np.float16)
        bq_col = np.ascontiguousarray(
            np.concatenate(bqc).reshape(128, 1))          # [128, 1]
        in_maps.append({
            "xT": xT, "wqkv": wq, "bq": bq_col,
            "wproj": wproj_16, "bproj": bproj_rs, "sel": sel,
        })
    return in_maps


def gather_out(results, B_=B, T_=T):
    SL = T_ // NCORES
    full = np.empty((B_, T_, D), np.float32)
    for c in range(NCORES):
        o = results[c]["out"]                            # [D, B_*SL]
        for b in range(B_):
            full[b, c * SL:(c + 1) * SL, :] = o[:, b * SL:(b + 1) * SL].T
    return full


_NC_CACHE = {}
LAST_EXEC_NS = None


def kernel(x, mask, W_qkv, b_qkv, W_proj, b_proj, trace=False):
    global LAST_EXEC_NS
    key = (B, T)
    if key not in _NC_CACHE:
        _NC_CACHE[key] = build_nc(B, T)
    nc = _NC_CACHE[key]
    in_maps = make_in_maps(x, W_qkv, b_qkv, W_proj, b_proj, B, T)
    res = run_bass_kernel_spmd(nc, in_maps, core_ids=list(range(NCORES)),
                               trace=trace)
    LAST_EXEC_NS = res.exec_time_ns
    return gather_out(res.results, B, T)


# revision 8
# speedup vs baseline: 1.3273x; 1.0420x over previous
"""Multi-head attention on 8 TRN2 NeuronCores (Bass/Tile, SPMD).

Sharding: tensor-parallel over heads (2 heads/core) for qkv + attention;
per-batch AllToAll to token-sharded layout (each core gets a 256-token
slice of every batch, all 16 heads) for the output projection.

Key structure (v2):
- qkv: x^T streamed in 1024-token tiles; q,k -> qT/kT [64dh x TOK] fp16 in
  SBUF (2 heads stacked on partitions); v transposed via PE into
  vn [ktok, vA|ones|vB] tiles so the PV matmul also emits softmax
  denominators (ones column shared by both heads).
- attention: scores computed transposed (S^T[k,q]); exp without max
  subtraction; causal handled by skipping k-tiles above the diagonal and
  shrinking matmuls/exp to the live q-range on diagonal tiles; inner loop
  software-pipelined (QK runs 2 tiles ahead of PV).
- normalization deferred past the collective: unnormalized PV rows plus
  denominator rows ship through a per-batch AllToAll [8,130,256]; the
  receiving core computes 1/denom once (ln/exp), broadcasts it across
  partitions with a tiny select-matmul, scales, projects, adds bias.
- k-bias dropped (softmax-invariant given q-bias); v-bias folded into the
  projection bias host-side.
"""

import math

import numpy as np

import concourse.bass as bass
import concourse.mybir as mybir
import concourse.tile as tile
from concourse import bacc
from concourse.bass_utils import run_bass_kernel_spmd
from concourse.masks import make_identity

# Route `exp` activations to the natural_log_exp_and_others table set so
# exp and ln share one ACT table load.
import concourse.bacc as _bacc_mod
from concourse.hw_specs import get_activation_tables as _orig_gat


def _gat_exp_with_ln(arch):
    d = dict(_orig_gat(arch))
    for name in d:
        if "exp" in name and "natural_log" not in name:
            d[name] = d[name] - {mybir.ActivationFunctionType.Exp}
    return d


_bacc_mod.get_activation_tables = _gat_exp_with_ln

# problem dims (fixed by the harness contract)
B, T, D, H = 4, 2048, 1024, 16
DH = D // H          # 64
NCORES = 8
HPC = H // NCORES    # 2 heads per core

F32 = mybir.dt.float32
FP16 = mybir.dt.float16
EXP = mybir.ActivationFunctionType.Exp
LOG = mybir.ActivationFunctionType.Ln


def emit(tc, io, B_, T_):
    nc = tc.nc
    TOK = B_ * T_
    CPB = T_ // 512           # 512-token compute chunks per batch
    NKT = T_ // 128           # k-tiles per batch
    SL = T_ // NCORES         # per-core token slice of each batch (256)

    xT, wqkv, bq, wproj, bproj, sel, out = (
        io["xT"], io["wqkv"], io["bq"], io["wproj"], io["bproj"],
        io["sel"], io["out"])

    with tc.tile_pool(name="consts", bufs=1) as consts, \
         tc.tile_pool(name="bigs", bufs=1) as bigs, \
         tc.tile_pool(name="dram", bufs=1, space="DRAM") as dram, \
         tc.tile_pool(name="xt", bufs=2) as xt_pool, \
         tc.tile_pool(name="vst", bufs=2) as vst_pool, \
         tc.tile_pool(name="ps", bufs=1, space="PSUM") as ps, \
         tc.tile_pool(name="expp", bufs=4) as ex_pool, \
         tc.tile_pool(name="attp", bufs=3) as at_pool, \
         tc.tile_pool(name="rhsp", bufs=2) as rhs_pool, \
         tc.tile_pool(name="denp", bufs=2) as den_pool, \
         tc.tile_pool(name="otp", bufs=3) as out_pool:

        # ---- constants (front: only what qkv batch 0 needs) ----
        w_sb = consts.tile([128, 8, 384], FP16)
        for d in range(8):
            nc.sync.dma_start(out=w_sb[:, d, :], in_=wqkv[d * 128:(d + 1) * 128, :])
        bq_sb = consts.tile([128, 1], F32)
        nc.sync.dma_start(out=bq_sb, in_=bq)
        sel_sb = consts.tile([16, 8, 128], FP16)
        nc.sync.dma_start(out=sel_sb, in_=sel)
        ident = consts.tile([128, 128], FP16)
        make_identity(nc, ident)
        # band masks: mask[j][pk, fq] = 1 if pk + 128*j <= fq else 0
        masks = consts.tile([128, 4, 512], FP16)
        nc.vector.memset(masks, 1.0)
        for j in range(4):
            nc.gpsimd.affine_select(
                out=masks[:, j, :], in_=masks[:, j, :],
                compare_op=mybir.AluOpType.is_ge, fill=0.0,
                base=-128 * j, pattern=[[1, 512]], channel_multiplier=-1)
        # deferred consts (needed from proj(0) on; loaded mid-stream)
        wproj_sb = consts.tile([128, 8, 1024], FP16)
        bproj_sb = consts.tile([128, 8], F32)

        # ---- big persistent buffers ----
        qT_sb = bigs.tile([128, TOK], FP16)
        kT_sb = bigs.tile([128, TOK], FP16)
        # vn: per k-tile [vA(64) | ones(1) | vB(64) | pad]
        vn_sb = bigs.tile([128, B_, NKT, 130], FP16)
        nc.vector.memset(vn_sb[:, :, :, 64:65], 1.0)

        a2a_in = [dram.tile([NCORES, 130, SL], FP16, name=f"a2ai{b}")
                  for b in range(B_)]
        a2a_out = [dram.tile([NCORES, 130, SL], FP16, name=f"a2ao{b}")
                   for b in range(B_)]

        def qkv_chunk(b, ci, xt_box):
            """One 512-token chunk of q/k/v for batch b."""
            ch = b * CPB + ci
            if ci % 2 == 0:
                xt_box[0] = xt_pool.tile([128, 8, 1024], FP16, tag="xt",
                                         name=f"xt{ch}")
                for d in range(8):
                    nc.sync.dma_start(
                        out=xt_box[0][:, d, :],
                        in_=xT[d * 128:(d + 1) * 128, ch * 512:(ch + 2) * 512])
            xt2 = xt_box[0]
            off = (ci % 2) * 512
            # q then k
            for ct, dst_is_q in ((0, True), (1, False)):
                qk = ps.tile([128, 512], F32, tag="qk", bufs=2,
                             name=f"qk{ch}_{ct}")
                for d in range(8):
                    nc.tensor.matmul(
                        qk, w_sb[:, d, ct * 128:(ct + 1) * 128],
                        xt2[:, d, off:off + 512],
                        start=(d == 0), stop=(d == 7))
                if dst_is_q:
                    nc.vector.tensor_scalar_add(
                        qT_sb[:, ch * 512:(ch + 1) * 512], qk, bq_sb[:, 0:1])
                else:
                    nc.vector.tensor_copy(
                        kT_sb[:, ch * 512:(ch + 1) * 512], qk)
            # v: [vdim, tok] -> SBUF fp16 -> PE transpose -> vn [tok, vdim]
            vp = ps.tile([128, 512], F32, tag="qk", bufs=2, name=f"vp{ch}")
            for d in range(8):
                nc.tensor.matmul(
                    vp, w_sb[:, d, 256:384], xt2[:, d, off:off + 512],
                    start=(d == 0), stop=(d == 7))
            vst = vst_pool.tile([128, 512], FP16, tag="vst", name=f"vst{ch}")
            nc.vector.tensor_copy(vst, vp)
            vtp = ps.tile([128, 1024], FP16, tag="vtp", bufs=1,
                          name=f"vtp{ch}")
            for sub in range(4):
                nc.tensor.transpose(
                    vtp[:, sub * 128:(sub + 1) * 128],
                    vst[:, sub * 128:(sub + 1) * 128], ident)
            t0 = ci * 4
            vtp4 = vtp[:, 0:512].rearrange("p (s x) -> p s x", s=4)
            nc.vector.tensor_copy(vn_sb[:, b, t0:t0 + 4, 0:64],
                                  vtp4[:, :, 0:64])
            nc.vector.tensor_copy(vn_sb[:, b, t0:t0 + 4, 65:129],
                                  vtp4[:, :, 64:128])

        def attention_unit(b, hh, qc):
            p0 = hh * 64
            nkt = 4 * qc + 4
            tb = b * T_

            def ncols(kt):
                return 512 - max(0, 128 * (kt - 4 * qc))

            q_ap = qT_sb[p0:p0 + 64, tb + qc * 512: tb + (qc + 1) * 512]
            pv = ps.tile([128, 512], F32, tag="pv", bufs=2,
                         name=f"pv{b}_{hh}_{qc}")
            scs = {}
            exs = {}

            def emit_qk(kt):
                n = ncols(kt)
                s = 512 - n
                sc = ps.tile([128, 512], F32, tag="sc", bufs=3,
                             name=f"sc{b}_{hh}_{qc}_{kt}")
                nc.tensor.matmul(
                    sc[:, s:512],
                    kT_sb[p0:p0 + 64, tb + kt * 128: tb + (kt + 1) * 128],
                    q_ap[:, s:512], start=True, stop=True)
                scs[kt] = sc

            emit_qk(0)
            if nkt > 1:
                emit_qk(1)
            for kt in range(nkt):
                n = ncols(kt)
                s = 512 - n
                ex = ex_pool.tile([128, 512], FP16, tag="ex",
                                  name=f"ex{b}_{hh}_{qc}_{kt}")
                nc.scalar.activation(ex[:, s:512], scs[kt][:, s:512], EXP,
                                     scale=1.0 / math.sqrt(DH))
                bj = kt - 4 * qc
                if bj >= 0:
                    nc.vector.tensor_mul(ex[:, s:512], ex[:, s:512],
                                         masks[:, bj, s:512])
                exs[kt] = ex
                del scs[kt]
                if kt + 2 < nkt:
                    emit_qk(kt + 2)
                vcol = hh * 64   # [vA|ones] or [ones|vB]
                nc.tensor.matmul(
                    pv[0:65, s:512], vn_sb[:, b, kt, vcol:vcol + 65],
                    ex[:, s:512], start=(kt == 0), stop=(kt == nkt - 1))
                del exs[kt]

            at = at_pool.tile([65, 512], FP16, tag="at",
                              name=f"at{b}_{hh}_{qc}")
            nc.vector.tensor_copy(at, pv[0:65, :])
            # rows: hh=0 -> data 0:64, denom 64; hh=1 -> denom 0, data 1:65
            drow = 64 if hh == 0 else 0
            d0 = 0 if hh == 0 else 1
            j0 = 2 * qc
            dst = a2a_in[b][j0:j0 + 2, p0:p0 + 64, :].rearrange(
                "j p c -> p j c")
            nc.sync.dma_start(
                out=dst,
                in_=at[d0:d0 + 64, :].rearrange("p (j c) -> p j c", j=2))
            ddst = a2a_in[b][j0:j0 + 2, 128 + hh:129 + hh, :].rearrange(
                "j one c -> one j c")
            nc.gpsimd.dma_start(
                out=ddst,
                in_=at[drow:drow + 1, :].rearrange("p (j c) -> p j c", j=2))

        def emit_a2a(b):
            nc.gpsimd.collective_compute(
                "AllToAll", mybir.AluOpType.bypass,
                replica_groups=[list(range(NCORES))],
                ins=[a2a_in[b][:]], outs=[a2a_out[b][:]])

        def proj_batch(b):
            rhs = rhs_pool.tile([128, 8, SL], FP16, tag="rhs",
                                name=f"rhs{b}")
            for i in range(8):
                nc.sync.dma_start(out=rhs[:, i, :], in_=a2a_out[b][i, 0:128, :])
            den = den_pool.tile([16, SL], FP16, tag="den", name=f"den{b}")
            for h in range(2):
                nc.sync.dma_start(out=den[h * 8:(h + 1) * 8, :],
                                  in_=a2a_out[b][:, 128 + h, :])
            d32 = den_pool.tile([16, SL], F32, tag="d32", name=f"d32{b}")
            nc.vector.tensor_copy(d32, den)
            r32 = den_pool.tile([16, SL], F32, tag="r32", name=f"r32{b}")
            nc.vector.reciprocal_approx_fast(r32, d32)
            rec = den_pool.tile([16, SL], FP16, tag="rec", name=f"rec{b}")
            nc.vector.tensor_copy(rec, r32)
            for i in range(8):
                rcb = ps.tile([128, 512], F32, tag="qk", bufs=2,
                              name=f"rcb{b}_{i}")
                nc.tensor.matmul(rcb[:, 0:SL], sel_sb[:, i, :], rec,
                                 start=True, stop=True)
                nc.vector.tensor_mul(rhs[:, i, :], rhs[:, i, :], rcb[:, 0:SL])
            for od in range(8):
                pj = ps.tile([128, 512], F32, tag="sc", bufs=3,
                             name=f"pj{b}_{od}")
                for i in range(8):
                    nc.tensor.matmul(
                        pj[:, 0:SL],
                        wproj_sb[:, i, od * 128:(od + 1) * 128],
                        rhs[:, i, :], start=(i == 0), stop=(i == 7))
                ot = out_pool.tile([128, SL], F32, tag="ot",
                                   name=f"ot{b}_{od}")
                nc.vector.tensor_scalar_add(ot, pj[:, 0:SL],
                                            bproj_sb[:, od:od + 1])
                nc.gpsimd.dma_start(
                    out=out[od * 128:(od + 1) * 128, b * SL:(b + 1) * SL],
                    in_=ot)

        # ---- schedule ----
        xt_box = [None]
        for ci in range(CPB):
            qkv_chunk(0, ci, xt_box)
        # deferred const loads (issue while batch-0 attention runs)
        for d in range(8):
            nc.sync.dma_start(out=wproj_sb[:, d, :],
                              in_=wproj[d * 128:(d + 1) * 128, :])
        nc.sync.dma_start(out=bproj_sb, in_=bproj)

        units = [(hh, qc) for hh in range(HPC) for qc in range(CPB)]
        for b in range(B_):
            for u, (hh, qc) in enumerate(units):
                attention_unit(b, hh, qc)
                if b + 1 < B_ and u % 2 == 1:
                    qkv_chunk(b + 1, u // 2, xt_box)
                if u == 5 and b >= 1:
                    proj_batch(b - 1)
            emit_a2a(b)
        proj_batch(B_ - 1)


def build_nc(B_=B, T_=T):
    TOK = B_ * T_
    SL = T_ // NCORES
    nc = bacc.Bacc("TRN2", target_bir_lowering=False, debug=False,
                   enable_asserts=False, num_devices=NCORES)
    io = {
        "xT": nc.dram_tensor("xT", [D, TOK], FP16, kind="ExternalInput").ap(),
        "wqkv": nc.dram_tensor("wqkv", [D, 3 * HPC * DH], FP16,
                               kind="ExternalInput").ap(),
        "bq": nc.dram_tensor("bq", [128, 1], F32, kind="ExternalInput").ap(),
        "wproj": nc.dram_tensor("wproj", [D, D], FP16,
                                kind="ExternalInput").ap(),
        "bproj": nc.dram_tensor("bproj", [128, 8], F32,
                                kind="ExternalInput").ap(),
        "sel": nc.dram_tensor("sel", [16, 8 * 128], FP16,
                              kind="ExternalInput").ap(),
        "out": nc.dram_tensor("out", [D, B_ * SL], F32,
                              kind="ExternalOutput").ap(),
    }
    io["sel"] = io["sel"].rearrange("p (i m) -> p i m", i=8)
    with tile.TileContext(nc) as tc:
        emit(tc, io, B_, T_)
    nc.compile()
    return nc


def make_in_maps(x, W_qkv, b_qkv, W_proj, b_proj, B_=B, T_=T):
    """Shard host inputs per core."""
    TOK = B_ * T_
    x2 = np.asarray(x, np.float32).reshape(TOK, D)
    xT = np.ascontiguousarray(x2.T).astype(np.float16)   # [D, TOK]
    W_qkv = np.asarray(W_qkv, np.float32)
    b_qkv = np.asarray(b_qkv, np.float32)
    W_proj = np.asarray(W_proj, np.float32)
    wproj_16 = W_proj.astype(np.float16)
    b_v = b_qkv[2 * D:3 * D]
    bproj_eff = np.asarray(b_proj, np.float32) + b_v @ W_proj
    bproj_rs = np.ascontiguousarray(bproj_eff.reshape(8, 128).T)  # [128, 8]
    # sel[q, i, m] = 1 iff q == (m // 64) * 8 + i
    sel = np.zeros((16, 8, 128), np.float16)
    for i in range(8):
        sel[i, i, 0:64] = 1.0
        sel[8 + i, i, 64:128] = 1.0
    sel = sel.reshape(16, 8 * 128)
    in_maps = []
    for c in range(NCORES):
        cols = []
        bqc = []
        for part in range(3):                            # q, k, v
            for h in (2 * c, 2 * c + 1):
                sl = slice(part * D + h * DH, part * D + (h + 1) * DH)
                cols.append(W_qkv[:, sl])
                if part == 0:
                    bqc.append(b_qkv[sl])
        wq = np.ascontiguousarray(np.concatenate(cols, axis=1)).astype(
            np.float16)
        bq_col = np.ascontiguousarray(
            np.concatenate(bqc).reshape(128, 1))          # [128, 1]
        in_maps.append({
            "xT": xT, "wqkv": wq, "bq": bq_col,
            "wproj": wproj_16, "bproj": bproj_rs, "sel": sel,
        })
    return in_maps


def gather_out(results, B_=B, T_=T):
    SL = T_ // NCORES
    full = np.empty((B_, T_, D), np.float32)
    for c in range(NCORES):
        o = results[c]["out"]                            # [D, B_*SL]
        for b in range(B_):
            full[b, c * SL:(c + 1) * SL, :] = o[:, b * SL:(b + 1) * SL].T
    return full


_NC_CACHE = {}
LAST_EXEC_NS = None


def kernel(x, mask, W_qkv, b_qkv, W_proj, b_proj, trace=False):
    global LAST_EXEC_NS
    key = (B, T)
    if key not in _NC_CACHE:
        _NC_CACHE[key] = build_nc(B, T)
    nc = _NC_CACHE[key]
    in_maps = make_in_maps(x, W_qkv, b_qkv, W_proj, b_proj, B, T)
    res = run_bass_kernel_spmd(nc, in_maps, core_ids=list(range(NCORES)),
                               trace=trace)
    LAST_EXEC_NS = res.exec_time_ns
    return gather_out(res.results, B, T)
